# revision 76
# baseline (speedup 1.0000x reference)
"""DGCNN classifier forward as a distributed Bass kernel on 8 TRN2 NeuronCores.

Sharding: pure data-parallel, one sample per core (B=8).

Per-core algorithm:
  - EdgeConv(x, W) decomposes: y[n,k] = Wa@x_j + (Wb-Wa)@x_i, so with
    u = x@Wa', w = x@(Wb-Wa)' + b (BN scale/bias folded host-side, scale>0),
    out[n] = leaky(max_k u[idx[n,k]] + w[n]); leaky(x) = max(0.2x, x) is a
    single DVE STT pass.
  - KNN: neg half-dist = A.T@B with A = [x; 1; -xx/2], B = [x; -xx/2-c; 1]
    on PE (c = 2^-40 keeps the self column away from denormals); the column
    id is planted in the low 10 mantissa bits (DVE STT with an iota tile).
    Topk is two-level: max8 per 128-col block (8 cheap passes) then 3x max8
    + 2x match_replace merge over the 64 candidates -- exact unless a block
    holds >8 of a point's true top-20 (68/32768 points here, 3.4e-3 rel-err
    contribution). Rank 1 is always self; ranks 2..20 are gathered via
    per-neighbor SWDGE indirect DMAs (the only SWDGE form walrus supports;
    500ns Pool floor each puts Pool at ~85% busy -- THE bottleneck), maxed
    by a progressive bf16 tensor-max tree that consumes gather rounds as
    they land.
  - Software pipelining hides everything else under the gather stream:
    per-tile u/w matmuls, squares, aux builds and u-table stores of EC i+1
    are emitted inside EC i's gather loop; cols 0-895 of the next EC's
    first dist+topk run early so only a 128-col sliver + merge sit on each
    boundary; x_t1/x_t2 pool units interleave as their point-tiles land
    (256-wide quarters keep f32r matmuls at full rate); p1's half of the
    classifier accumulates mid-loop. Warm-up matmuls keep the PE p-state
    ramped through the start-of-kernel load window and the last gather
    window (idle resets matmuls to half rate). Const-setup and aux copies
    run on DVE, keeping the ACT queue free for the sq/dist copies that
    gate each topk.
  - nf1 = [xt1,xt1] duplication: distances are 2x those of xt1 (same topk
    order) and the 512-wide conv folds to 256-wide (W halves summed).
  - x_t1/x_t2 only feed max-pools: computed channel-major, leaky after max.

Cost-model time: 347923 ns/core (baseline 404786), rel err 1.21e-2.
"""

import math
import numpy as np

import concourse.bass as bass
import concourse.mybir as mybir
import concourse.tile as tile
from concourse.bass_utils import run_bass_kernel_spmd
from concourse.masks import make_identity

F32 = mybir.dt.float32
F32R = mybir.dt.float32r
U32 = mybir.dt.uint32
BF16 = mybir.dt.bfloat16
AX = mybir.AxisListType
OP = mybir.AluOpType
ACTF = mybir.ActivationFunctionType

P = 128
N = 1024
NT = N // P          # 8 point tiles
KNN = 20
NEG = -3.0e38


def build_nc():
    nc = bass.Bass()

    def _in(name, shape):
        # weights are pre-rounded to f32r bit patterns on the host
        return nc.declare_dram_parameter(name, shape, F32R, isOutput=False)

    xs = nc.declare_dram_parameter("xs", [3, N], F32R, isOutput=False)
    uW1 = _in("uW1", [3, 64]); wW1 = _in("wW1", [3, 64]); wb1 = _in("wb1", [1, 64])
    uW2 = _in("uW2", [64, 64]); wW2 = _in("wW2", [64, 64]); wb2 = _in("wb2", [1, 64])
    uW3 = _in("uW3", [128, 256]); wW3 = _in("wW3", [128, 256]); wb3 = _in("wb3", [1, 256])
    uW4 = _in("uW4", [256, 256]); wW4 = _in("wW4", [256, 256]); wb4 = _in("wb4", [1, 256])
    w2m = _in("w2m", [128, N]); w2mb = _in("w2mb", [1, N])
    w5 = _in("w5", [512, N]); w5b = _in("w5b", [1, N])
    l1 = nc.declare_dram_parameter("l1", [2048, 512], BF16, isOutput=False)
    l1b = _in("l1b", [1, 512])
    l2 = nc.declare_dram_parameter("l2", [512, 256], BF16, isOutput=False)
    l2b = _in("l2b", [1, 256])
    l3 = nc.declare_dram_parameter("l3", [256, 40], BF16, isOutput=False)
    l3b = _in("l3b", [1, 40])
    out_d = nc.declare_dram_parameter("out", [1, 40], F32, isOutput=True)

    with tile.TileContext(nc) as tc:
        with (
            tc.tile_pool(name="const", bufs=1) as const,
            tc.tile_pool(name="wpool", bufs=1) as wp,
            tc.tile_pool(name="base", bufs=1) as bp,
            tc.tile_pool(name="sq", bufs=1) as sqp,
            tc.tile_pool(name="dist", bufs=6) as dp,
            tc.tile_pool(name="vals", bufs=6) as vp,
            tc.tile_pool(name="idx", bufs=6) as ip,
            tc.tile_pool(name="uw", bufs=1) as uwp,
            tc.tile_pool(name="gat", bufs=3) as gp,
            tc.tile_pool(name="xout", bufs=1) as xp,
            tc.tile_pool(name="mlp", bufs=8) as mp,
            tc.tile_pool(name="misc", bufs=1) as mc,
            tc.tile_pool(name="lk", bufs=1) as lkp,
            tc.tile_pool(name="ps", bufs=7, space="PSUM") as pp,
            tc.tile_pool(name="dram", bufs=1, space="DRAM") as dr,
        ):
            def leaky(out, in_):
                # leaky(x) = max(0.2x, x): a single DVE STT pass. PSUM
                # sources may only be read once per instruction, so those
                # fall back to ACT relu(-0.8x) + DVE add.
                if in_.space == bass.MemorySpace.PSUM:
                    shp = list(in_.shape)
                    fs = in_.free_size()
                    rt = lkp.tile([1, 512], in_.dtype, tag="lk1f", bufs=1,
                                  name=f"lk_{nc.next_id()}")
                    rv = rt[:shp[0], :fs]
                    nc.scalar.activation(out=rv, in_=in_, func=ACTF.Relu,
                                         scale=-0.8)
                    nc.vector.tensor_add(out=out, in0=in_, in1=rv)
                else:
                    nc.vector.scalar_tensor_tensor(
                        out=out, in0=in_, scalar=0.2, in1=in_,
                        op0=OP.mult, op1=OP.max)

            ident = const.tile([P, P], F32)
            make_identity(nc, ident[:])
            identb = const.tile([P, P], BF16)
            nc.vector.tensor_copy(out=identb[:], in_=ident[:])
            # memset can't emit f32r: init an f32 twin, ACT-copy (casts)
            cscr = const.tile([P, 4], F32)    # cols: [1, 0, -0.5, -2^-40]
            nc.vector.memset(cscr[:, 0:1], 1.0)
            nc.vector.memset(cscr[:, 1:2], 0.0)
            nc.vector.memset(cscr[:, 2:3], -0.5)
            nc.vector.memset(cscr[:, 3:4], -(2.0 ** -40))
            # index-in-value topk constants: distances are quantized to 13
            # mantissa bits and the column id is planted in the low 10 bits,
            # so max8 winners carry their own indices (no max_index passes).
            iotac = const.tile([P, N], U32)
            nc.gpsimd.iota(iotac[:], pattern=[[1, N]], base=0,
                           channel_multiplier=0)
            maskcol = const.tile([P, 1], U32)
            nc.gpsimd.iota(maskcol[:], pattern=[[0, 1]], base=0xFFFFFC00,
                           channel_multiplier=0)
            andcol = const.tile([P, 1], U32)
            nc.gpsimd.iota(andcol[:], pattern=[[0, 1]], base=0x3FF,
                           channel_multiplier=0)
            ones1f = const.tile([1, N], F32)
            nc.vector.memset(ones1f[:], 1.0)
            ones1 = const.tile([1, N], F32R)
            nc.vector.tensor_copy(out=ones1[:], in_=ones1f[:])
            znA = const.tile([P, 2], F32R)    # cols [0,-1]
            nc.vector.tensor_copy(out=znA[:], in_=cscr[:, 1:3])
            znB = const.tile([P, 2], F32R)    # cols [-1,0]
            nc.vector.tensor_copy(out=znB[:, 0:1], in_=cscr[:, 2:3])
            nc.vector.tensor_copy(out=znB[:, 1:2], in_=cscr[:, 1:2])
            oA = const.tile([1, 2], F32R)     # [1,0]
            nc.vector.tensor_copy(out=oA[:], in_=cscr[:1, 0:2])
            # oB = [-2^-40, 1]: shifts every distance by -2^-40 so the self
            # column (dist 0) encodes as a normal float, never a denormal
            oB = const.tile([1, 2], F32R)
            nc.vector.tensor_copy(out=oB[:, 0:1], in_=cscr[:1, 3:4])
            nc.vector.tensor_copy(out=oB[:, 1:2], in_=cscr[:1, 0:1])

            # ramp warm-up: run the PE during the x-load window so the
            # first aux/dist matmuls start at full p-state, not cold
            wrm0 = pp.tile([P, P], F32, tag="ps", name="wrm0")
            for _ in range(5):
                nc.tensor.matmul(wrm0[:], ident[:, 0:P], ident[:],
                                 start=True, stop=True)

            # ---------- weights ----------
            def _load(dram, shape, eng=None):
                t = wp.tile(shape, F32R, tag=dram.name, name=f"{dram.name}_s")
                (eng or nc.gpsimd).dma_start(out=t[:], in_=dram[:])
                return t

            # the point cloud loads FIRST: it heads the ramp's serial chain
            # (sq -> aux -> dist -> topk); ec1's weights only feed psu/psw
            # which run later
            base0 = bp.tile([3, N], F32R, tag="base0")
            nc.sync.dma_start(out=base0[:3, 0:512], in_=xs[:, 0:512])
            nc.sync.dma_start(out=base0[:3, 512:], in_=xs[:, 512:])
            uW1s = _load(uW1, [3, 64], nc.sync); wW1s = _load(wW1, [3, 64], nc.sync)
            wb1s = _load(wb1, [1, 64], nc.sync)
            u1d = dr.tile([N, 64], BF16)
            u2d = dr.tile([N, 64], BF16)
            u3d = dr.tile([N, 256], BF16)
            u4d = dr.tile([N, 256], BF16)

            # ---------- helpers ----------
            def cs(c):
                return slice(c * P, (c + 1) * P)

            def dist_half(lhs_ktiles, rhs_ktiles, d, c, h):
                hs = slice(h * 512, (h + 1) * 512)
                psd = pp.tile([P, 512], F32, tag="ps",
                              name=f"psd_{nc.next_id()}")
                nk = len(lhs_ktiles)
                for ki, (lf, rf) in enumerate(zip(lhs_ktiles, rhs_ktiles)):
                    nc.tensor.matmul(psd[:], lf(c), rf(hs),
                                     start=(ki == 0), stop=(ki == nk - 1))
                nc.scalar.copy(out=d[:, hs], in_=psd[:])

            def dist_tiles(lhs_ktiles, rhs_ktiles, tiles=None):
                out = []
                for c in (range(NT) if tiles is None else tiles):
                    d = dp.tile([P, N], F32, tag="dist", name=f"d_{nc.next_id()}")
                    dist_half(lhs_ktiles, rhs_ktiles, d, c, 0)
                    dist_half(lhs_ktiles, rhs_ktiles, d, c, 1)
                    out.append(d)
                return out

            def topk_half(d, v64, h):
                # encode: plant the column id in the low 10 mantissa bits
                # (DVE only: TensorScalarPtr is illegal on the Pool engine),
                # then max8 each 128-col block (8 cheap passes over 128
                # elems instead of 3+2 passes over 1024)
                hs = slice(h * 512, (h + 1) * 512)
                nc.vector.scalar_tensor_tensor(
                    out=d[:, hs].bitcast(U32), in0=d[:, hs].bitcast(U32),
                    scalar=maskcol[:], in1=iotac[:, hs],
                    op0=OP.bitwise_and, op1=OP.bitwise_or)
                for blk in range(4):
                    b = 4 * h + blk
                    nc.vector.max(out=v64[:, b * 8:(b + 1) * 8],
                                  in_=d[:, b * 128:(b + 1) * 128])

            def topk_merge(v64):
                # top-20 of the 64 block candidates: 3x max8 + 2x
                # match_replace on [P, 64]. Exact unless a block holds >8
                # of a point's true top-20 (measured: 68/32768 points on
                # this dataset, end-to-end rel err contribution 3.4e-3).
                # Rank 1 is always the self column (its shifted distance
                # -2^-40 beats every real distance).
                v = vp.tile([P, 24], F32, tag="vals", name=f"v_{nc.next_id()}")
                idx24 = ip.tile([P, 24], U32, tag="idx", name=f"idx_{nc.next_id()}")
                for r in range(3):
                    rs = slice(r * 8, (r + 1) * 8)
                    nc.vector.max(out=v[:, rs], in_=v64[:])
                    if r < 2:
                        nc.vector.match_replace(
                            out=v64[:], in_to_replace=v[:, rs],
                            in_values=v64[:], imm_value=NEG)
                    # per-round decode so the first gathers can launch after
                    # round 1 instead of waiting for the whole topk
                    nc.vector.tensor_scalar(out=idx24[:, rs],
                                            in0=v[:, rs].bitcast(U32),
                                            scalar1=andcol[:], scalar2=None,
                                            op0=OP.bitwise_and)
                return idx24

            def topk20(d, split_encode=False):
                v64 = vp.tile([P, 64], F32, tag="cand", bufs=4,
                              name=f"c_{nc.next_id()}")
                topk_half(d, v64, 0)
                topk_half(d, v64, 1)
                return topk_merge(v64)

            def gather_combine(idx24, ud, w_ap, C, c, xtile, uall):
                """xtile = leaky(max_k u[idx] + w).

                Rank-1 is always self, whose u-row already sits in this
                partition of u_all — ACT-copy it into slot 0. Ranks 2..20
                come via 19 per-column indirect gathers (one offset per
                partition — the only SWDGE form walrus descriptor-gen
                supports; multi-offset APs scramble), contiguous [P, C]
                dest slices of one wide tile."""
                big = gp.tile([P, KNN * C], BF16, tag="gat", bufs=3,
                              name=f"g_{nc.next_id()}")
                nc.scalar.copy(out=big[:, 0:C], in_=uall[:, c, :])
                for t in range(1, KNN):
                    nc.gpsimd.indirect_dma_start(
                        out=big[:, t * C:(t + 1) * C], out_offset=None,
                        in_=ud[:],
                        in_offset=bass.IndirectOffsetOnAxis(
                            ap=idx24[:, t:t + 1], axis=0),
                    )
                # progressive bf16 TT-max tree (2-byte packed -> DVE 2x
                # mode): ordered so everything except one C-wide fold is
                # done before the LAST gather chunk lands -- only
                # max+add+leaky remain on the post-gather critical path
                nc.vector.tensor_max(out=big[:, :4 * C], in0=big[:, :4 * C],
                                     in1=big[:, 4 * C:8 * C])
                nc.vector.tensor_max(out=big[:, :4 * C], in0=big[:, :4 * C],
                                     in1=big[:, 8 * C:12 * C])
                nc.vector.tensor_max(out=big[:, :4 * C], in0=big[:, :4 * C],
                                     in1=big[:, 12 * C:16 * C])
                nc.vector.tensor_max(out=big[:, :2 * C], in0=big[:, :2 * C],
                                     in1=big[:, 16 * C:18 * C])
                nc.vector.tensor_max(out=big[:, :2 * C], in0=big[:, :2 * C],
                                     in1=big[:, 2 * C:4 * C])
                nc.vector.tensor_max(out=big[:, :C], in0=big[:, :C],
                                     in1=big[:, C:2 * C])
                nc.vector.tensor_max(out=big[:, :C], in0=big[:, :C],
                                     in1=big[:, 18 * C:19 * C])
                nc.vector.tensor_max(out=big[:, :C], in0=big[:, :C],
                                     in1=big[:, 19 * C:20 * C])
                m = uwp.tile([P, C], BF16, tag="m", bufs=3,
                             name=f"m_{nc.next_id()}")
                nc.vector.tensor_add(out=m[:], in0=big[:, :C], in1=w_ap)
                leaky(xtile[:], m[:])

            def build_aux(sq_aps, auxA, auxB):
                """auxA = [ones; -xx], auxB = [-xx; ones] via PE rank updates."""
                for h in range(2):
                    build_aux_h(sq_aps, auxA, auxB, h)

            def build_aux_cols(sq_aps, auxA, auxB, lo, hi):
                # auxB first: the next dist half's rhs needs it immediately,
                # while auxA's lhs col-slice is only read per point-tile
                for aux, orow, zn in ((auxB, oB, znB), (auxA, oA, znA)):
                    psa = pp.tile([2, hi - lo], F32, tag="ps",
                                  name=f"psa_{nc.next_id()}")
                    nc.tensor.matmul(psa[:], orow[:1, :2], ones1[:1, lo:hi],
                                     start=True, stop=False)
                    for i, sqa in enumerate(sq_aps):
                        kp = sqa.shape[0]
                        nc.tensor.matmul(psa[:], zn[:kp, :2], sqa[:, lo:hi],
                                         start=False,
                                         stop=(i == len(sq_aps) - 1))
                    nc.vector.tensor_copy(out=aux[0:2, lo:hi], in_=psa[:])

            def build_aux_h(sq_aps, auxA, auxB, h):
                build_aux_cols(sq_aps, auxA, auxB, h * 512, (h + 1) * 512)

            def transpose_into(src_ap, dst_tile, dst_row0, c):
                w = src_ap.shape[1]
                pst = pp.tile([w, P], src_ap.dtype, tag="ps",
                              name=f"pst_{nc.next_id()}")
                idt = identb if src_ap.dtype == BF16 else ident
                nc.tensor.transpose(out=pst[:], in_=src_ap, identity=idt[:])
                nc.scalar.copy(out=dst_tile[dst_row0:dst_row0 + w, cs(c)],
                               in_=pst[:])

            def store_u_h(u_all, ud, h):
                # half stores so each can launch after its 4 psu copies,
                # pulling the store off the first-gather critical path
                half = N // 2
                nc.sync.dma_start(
                    out=ud[h * half:(h + 1) * half].rearrange(
                        "(c p) f -> p c f", p=P),
                    in_=u_all[:, h * 4:(h + 1) * 4, :])

            def store_u(u_all, ud):
                store_u_h(u_all, ud, 0)
                store_u_h(u_all, ud, 1)

            # =====================================================
            # ec1
            # =====================================================
            auxA0 = bp.tile([2, N], F32R, tag="auxA", bufs=2)
            auxB0 = bp.tile([2, N], F32R, tag="auxB", bufs=2)
            sq0 = sqp.tile([3, N], F32R, tag="sq", bufs=2, name="sq0")
            nc.scalar.square(out=sq0[:, 0:512], in_=base0[:3, 0:512])
            nc.scalar.square(out=sq0[:, 512:], in_=base0[:3, 512:])
            build_aux([sq0[:3, :]], auxA0, auxB0)

            x1 = [xp.tile([P, 64], BF16, tag=f"x1_{c}", bufs=2, name=f"x1_{c}")
                  for c in range(NT)]
            u_all1 = uwp.tile([P, NT, 64], BF16, tag="uall64", bufs=2, name="ua1")
            for c in range(NT):
                psu = pp.tile([P, 64], F32, tag="ps", name=f"psu1_{c}")
                nc.tensor.matmul(psu[:], base0[:3, cs(c)], uW1s[:])
                nc.scalar.copy(out=u_all1[:, c, :], in_=psu[:])
                if c in (3, NT - 1):
                    store_u_h(u_all1, u1d, 0 if c == 3 else 1)
            lhs1 = [lambda c: base0[:3, cs(c)], lambda c: auxA0[0:2, cs(c)]]
            rhs1 = [lambda hs: base0[:3, hs], lambda hs: auxB0[0:2, hs]]
            d1 = dist_tiles(lhs1, rhs1, tiles=[0])
            w_sb1 = uwp.tile([P, NT, 64], BF16, tag="wall64", bufs=2,
                             name="wsb1")
            for c in range(NT):
                psw = pp.tile([P, 64], F32, tag="ps", name=f"psw1_{c}")
                nc.tensor.matmul(psw[:], base0[:3, cs(c)], wW1s[:],
                                 start=True, stop=False)
                nc.tensor.matmul(psw[:], ones1[:1, cs(c)], wb1s[:],
                                 start=False, stop=True)
                nc.scalar.copy(out=w_sb1[:, c, :], in_=psw[:])
            d1 += dist_tiles(lhs1, rhs1, tiles=range(1, NT))
            base1 = bp.tile([P, N], F32R, tag="base1")
            # bulk weight loads for ec2..mlp on the SP queue (off Pool; SP
            # drains all of this during ec1's gather phase)
            uW2s = _load(uW2, [64, 64], nc.sync); wW2s = _load(wW2, [64, 64], nc.sync)
            wb2s = _load(wb2, [1, 64], nc.sync)
            uW3s = _load(uW3, [128, 256], nc.sync); wW3s = _load(wW3, [128, 256], nc.sync)
            wb3s = _load(wb3, [1, 256], nc.sync); wb4s = _load(wb4, [1, 256], nc.sync)
            uW4s = [wp.tile([P, 256], F32R, tag=f"uW4_{i}", name=f"uW4s_{i}")
                    for i in range(2)]
            wW4s = [wp.tile([P, 256], F32R, tag=f"wW4_{i}", name=f"wW4s_{i}")
                    for i in range(2)]
            for i in range(2):
                nc.sync.dma_start(out=uW4s[i][:], in_=uW4[i * P:(i + 1) * P, :])
                nc.sync.dma_start(out=wW4s[i][:], in_=wW4[i * P:(i + 1) * P, :])
            w2ms = _load(w2m, [128, N], nc.sync)
            p1bias = wp.tile([P, NT], F32R, tag="p1bias", name="p1bias")
            nc.sync.dma_start(
                out=p1bias[:], in_=w2mb[:].rearrange("o (j p) -> (o p) j", p=P))
            w5s = [wp.tile([P, N], F32R, tag=f"w5_{i}", name=f"w5s_{i}")
                   for i in range(4)]
            for i in range(4):
                nc.sync.dma_start(out=w5s[i][:], in_=w5[i * P:(i + 1) * P, :])
            p2bias = wp.tile([P, NT], F32R, tag="p2bias", name="p2bias")
            nc.sync.dma_start(
                out=p2bias[:], in_=w5b[:].rearrange("o (j p) -> (o p) j", p=P))
            l1bs = _load(l1b, [1, 512], nc.sync); l2bs = _load(l2b, [1, 256], nc.sync)
            l3bs = _load(l3b, [1, 40], nc.sync)
            # prefetch the classifier weights on the SP queue too (it drains
            # all of this during ec1, well before its next dependent op)
            l1ts = [mp.tile([P, 512], BF16, tag=f"l1_{j % 12}", bufs=1,
                            name=f"l1t_{j}") for j in range(16)]
            for j in range(12):
                nc.sync.dma_start(out=l1ts[j][:], in_=l1[j * P:(j + 1) * P, :])
            l2ts = [mp.tile([P, 256], BF16, tag=f"l2_{j}", bufs=1, name=f"l2t_{j}")
                    for j in range(4)]
            for j in range(4):
                nc.sync.dma_start(out=l2ts[j][:], in_=l2[j * P:(j + 1) * P, :])
            l3ts = [mp.tile([P, 40], BF16, tag=f"l3_{j}", bufs=1, name=f"l3t_{j}")
                    for j in range(2)]
            for j in range(2):
                nc.sync.dma_start(out=l3ts[j][:], in_=l3[j * P:(j + 1) * P, :])

            # ec2 prep, hoisted into ec1's gather loop: per-tile u/w matmuls
            # as soon as tile c of base1 is transposed, sq/aux/u-store per
            # half at c==3/7 — so the ec1->ec2 boundary only has to run
            # dist+topk of ec2's first tile before its gathers start.
            auxA2 = bp.tile([2, N], F32R, tag="auxA", bufs=2)
            auxB2 = bp.tile([2, N], F32R, tag="auxB", bufs=2)
            sq2 = sqp.tile([64, N], F32R, tag="sq", bufs=2, name="sq2")
            u_all2 = uwp.tile([P, NT, 64], BF16, tag="uall64", bufs=2, name="ua2")
            w_sb2 = uwp.tile([P, NT, 64], BF16, tag="wall64", bufs=2,
                             name="wsb2")
            lhs2 = [lambda c: base1[:64, cs(c)], lambda c: auxA2[0:2, cs(c)]]
            rhs2 = [lambda hs: base1[:64, hs], lambda hs: auxB2[0:2, hs]]
            early = {}

            def dist_cols(lhs_ktiles, rhs_ktiles, d, c, lo, hi):
                # dist over a column sub-range [lo, hi) of tile c
                psd = pp.tile([P, hi - lo], F32, tag="ps",
                              name=f"psd_{nc.next_id()}")
                nk = len(lhs_ktiles)
                for ki, (lf, rf) in enumerate(zip(lhs_ktiles, rhs_ktiles)):
                    nc.tensor.matmul(psd[:], lf(c), rf(slice(lo, hi)),
                                     start=(ki == 0), stop=(ki == nk - 1))
                nc.scalar.copy(out=d[:, lo:hi], in_=psd[:])

            def topk_blocks(d, v64, blo, bhi):
                # encode + block max8 for 128-col blocks [blo, bhi)
                nc.vector.scalar_tensor_tensor(
                    out=d[:, blo * 128:bhi * 128].bitcast(U32),
                    in0=d[:, blo * 128:bhi * 128].bitcast(U32),
                    scalar=maskcol[:], in1=iotac[:, blo * 128:bhi * 128],
                    op0=OP.bitwise_and, op1=OP.bitwise_or)
                for b in range(blo, bhi):
                    nc.vector.max(out=v64[:, b * 8:(b + 1) * 8],
                                  in_=d[:, b * 128:(b + 1) * 128])

            def early_t0(key, lhs, rhs):
                # cols 0-511 of the next EC's first dist+topk, emitted while
                # the current EC's gathers still stream (needs aux h0 +
                # output tiles 0-3 only)
                d = dp.tile([P, N], F32, tag="dist", name=f"d_{nc.next_id()}")
                v64 = vp.tile([P, 64], F32, tag="cand", bufs=4,
                              name=f"c_{nc.next_id()}")
                dist_half(lhs, rhs, d, 0, 0)
                topk_half(d, v64, 0)
                early[key] = (d, v64)

            def early_t0_mid(key, lhs, rhs, sqs, auxA, auxB):
                # cols 512-895: aux/dist/encode over everything but the
                # last 128-col sliver, emitted once output tiles 4-6 exist
                for sqa, base_ap in sqs:
                    nc.scalar.square(out=sqa[:, 512:896], in_=base_ap[:, 512:896])
                build_aux_cols([s for s, _ in sqs], auxA, auxB, 512, 896)
                d, v64 = early[key]
                dist_cols(lhs, rhs, d, 0, 512, 896)
                topk_blocks(d, v64, 4, 7)

            def finish_t0(key, lhs, rhs, sqs, auxA, auxB):
                # the last 128-col sliver + merge: the only topk work left
                # on the boundary critical path
                for sqa, base_ap in sqs:
                    nc.scalar.square(out=sqa[:, 896:], in_=base_ap[:, 896:])
                build_aux_cols([s for s, _ in sqs], auxA, auxB, 896, 1024)
                d, v64 = early[key]
                dist_cols(lhs, rhs, d, 0, 896, 1024)
                topk_blocks(d, v64, 7, 8)
                return d, topk_merge(v64)

            def prep2(c):
                # at the last tile, sq/aux jump ahead of psu/psw on the
                # ACT queue: they gate the next EC's first dist+topk half
                psu = pp.tile([P, 64], F32, tag="ps", name=f"psu2_{c}")
                nc.tensor.matmul(psu[:], base1[:64, cs(c)], uW2s[:])
                nc.scalar.copy(out=u_all2[:, c, :], in_=psu[:])
                psw = pp.tile([P, 64], F32, tag="ps", name=f"psw2_{c}")
                nc.tensor.matmul(psw[:], base1[:64, cs(c)], wW2s[:],
                                 start=True, stop=False)
                nc.tensor.matmul(psw[:], ones1[:1, cs(c)], wb2s[:],
                                 start=False, stop=True)
                nc.scalar.copy(out=w_sb2[:, c, :], in_=psw[:])
                if c == 3:
                    nc.scalar.square(out=sq2[:, 0:512], in_=base1[:64, 0:512])
                    build_aux_h([sq2[:64, :]], auxA2, auxB2, 0)
                    store_u_h(u_all2, u2d, 0)
                elif c == NT - 1:
                    store_u_h(u_all2, u2d, 1)
                elif c == 4:
                    early_t0("ec2", lhs2, rhs2)
                elif c == 6:
                    early_t0_mid("ec2", lhs2, rhs2,
                                 [(sq2[:64, :], base1[:64, :])], auxA2, auxB2)

            idxs1 = [topk20(d1[0], split_encode=True)]
            for c in range(NT):
                if c + 1 < NT:
                    idxs1.append(topk20(d1[c + 1]))
                gather_combine(idxs1[c], u1d, w_sb1[:, c, :], 64, c, x1[c], u_all1)
                transpose_into(x1[c][:, :64], base1, 0, c)
                prep2(c)

            # =====================================================
            # ec2
            # =====================================================
            x2 = [xp.tile([P, 64], BF16, tag=f"x1_{c}", bufs=2, name=f"x2_{c}")
                  for c in range(NT)]
            d2_t0, idx2_t0 = finish_t0("ec2", lhs2, rhs2,
                [(sq2[:64, :], base1[:64, :])], auxA2, auxB2)
            d2 = [d2_t0]
            d2 += dist_tiles(lhs2, rhs2, tiles=range(1, NT))

            # ec3 prep (hoisted into ec2's loop); ec3 runs on the 128-dim
            # xt1 = [x1; x2], so tile c is ready after ec2's transpose c
            auxA3 = bp.tile([2, N], F32R, tag="auxA", bufs=2)
            auxB3 = bp.tile([2, N], F32R, tag="auxB", bufs=2)
            sq3 = sqp.tile([P, N], F32R, tag="sq", bufs=2, name="sq3")
            u_all3 = uwp.tile([P, NT, 256], BF16, tag="uall256", bufs=2, name="ua3")
            w_sb3 = uwp.tile([P, NT, 256], BF16, tag="wall256", bufs=2,
                             name="wsb3")
            lhs3 = [lambda c: base1[:, cs(c)], lambda c: auxA3[0:2, cs(c)]]
            rhs3 = [lambda hs: base1[:, hs], lambda hs: auxB3[0:2, hs]]

            def prep3(c):
                psu = pp.tile([P, 256], F32, tag="ps", name=f"psu3_{c}")
                nc.tensor.matmul(psu[:], base1[:, cs(c)], uW3s[:])
                nc.scalar.copy(out=u_all3[:, c, :], in_=psu[:])
                psw = pp.tile([P, 256], F32, tag="ps", name=f"psw3_{c}")
                nc.tensor.matmul(psw[:], base1[:, cs(c)], wW3s[:],
                                 start=True, stop=False)
                nc.tensor.matmul(psw[:], ones1[:1, cs(c)], wb3s[:],
                                 start=False, stop=True)
                nc.scalar.copy(out=w_sb3[:, c, :], in_=psw[:])
                if c == 3:
                    nc.scalar.square(out=sq3[:, 0:512], in_=base1[:, 0:512])
                    build_aux_h([sq3[:, :]], auxA3, auxB3, 0)
                    store_u_h(u_all3, u3d, 0)
                elif c == NT - 1:
                    store_u_h(u_all3, u3d, 1)
                elif c == 4:
                    early_t0("ec3", lhs3, rhs3)
                elif c == 6:
                    early_t0_mid("ec3", lhs3, rhs3,
                                 [(sq3[:, :], base1[:, :])], auxA3, auxB3)

            idxs2 = [idx2_t0]
            for c in range(NT):
                if c + 1 < NT:
                    idxs2.append(topk20(d2[c + 1]))
                gather_combine(idxs2[c], u2d, w_sb2[:, c, :], 64, c, x2[c], u_all2)
                transpose_into(x2[c][:, :64], base1, 64, c)
                prep3(c)

            # =====================================================
            # ec3 (on 128-dim xt1)
            # =====================================================
            x3 = [xp.tile([P, 256], BF16, tag=f"x3_{c}", bufs=2, name=f"x3_{c}")
                  for c in range(NT)]
            # x_t1 -> p1 units (channel-major, leaky after max), interleaved
            # into ec3's gather phase where PE/DVE have slack
            fcol = const.tile([P, 16], F32R)
            p1tmp = mc.tile([P, 16], F32, tag="ptmp", name="p1tmp")

            def p1_unit(j, h):
                hs = slice(h * 512, (h + 1) * 512)
                pst = pp.tile([P, 512], F32, tag="ps", name=f"pt1_{j}_{h}")
                nc.tensor.matmul(pst[:], w2ms[:, cs(j)], base1[:, hs])
                nc.vector.tensor_reduce(
                    out=p1tmp[:, 2 * j + h:2 * j + h + 1], in_=pst[:],
                    axis=AX.X, op=OP.max)

            d3_t0, idx3_t0 = finish_t0("ec3", lhs3, rhs3,
                [(sq3[:, :], base1[:, :])], auxA3, auxB3)
            d3 = [d3_t0]
            d3 += dist_tiles(lhs3, rhs3, tiles=range(1, NT))
            base3 = [bp.tile([P, N], F32R, tag=f"base3_{i}", name=f"base3_{i}")
                     for i in range(2)]

            # ec4 prep (hoisted into ec3's gather loop)
            auxA4 = bp.tile([2, N], F32R, tag="auxA", bufs=2)
            auxB4 = bp.tile([2, N], F32R, tag="auxB", bufs=2)
            sq4a = sqp.tile([P, N], F32R, tag="sq", bufs=2, name="sq4a")
            sq4b = sqp.tile([P, N], F32R, tag="sq", bufs=2, name="sq4b")
            u_all4 = uwp.tile([P, NT, 256], BF16, tag="uall256", bufs=2, name="ua4")
            w_sb4 = uwp.tile([P, NT, 256], BF16, tag="wall256", bufs=2,
                             name="wsb4")
            lhs4 = [lambda c: base3[0][:, cs(c)], lambda c: base3[1][:, cs(c)],
                    lambda c: auxA4[0:2, cs(c)]]
            rhs4 = [lambda hs: base3[0][:, hs], lambda hs: base3[1][:, hs],
                    lambda hs: auxB4[0:2, hs]]

            def prep4(c):
                psu = pp.tile([P, 256], F32, tag="ps", name=f"psu4_{c}")
                nc.tensor.matmul(psu[:], base3[0][:, cs(c)], uW4s[0][:],
                                 start=True, stop=False)
                nc.tensor.matmul(psu[:], base3[1][:, cs(c)], uW4s[1][:],
                                 start=False, stop=True)
                nc.scalar.copy(out=u_all4[:, c, :], in_=psu[:])
                psw = pp.tile([P, 256], F32, tag="ps", name=f"psw4_{c}")
                nc.tensor.matmul(psw[:], base3[0][:, cs(c)], wW4s[0][:],
                                 start=True, stop=False)
                nc.tensor.matmul(psw[:], base3[1][:, cs(c)], wW4s[1][:],
                                 start=False, stop=False)
                nc.tensor.matmul(psw[:], ones1[:1, cs(c)], wb4s[:],
                                 start=False, stop=True)
                nc.scalar.copy(out=w_sb4[:, c, :], in_=psw[:])
                if c == 3:
                    nc.scalar.square(out=sq4a[:, 0:512], in_=base3[0][:, 0:512])
                    nc.scalar.square(out=sq4b[:, 0:512], in_=base3[1][:, 0:512])
                    build_aux_h([sq4a[:, :], sq4b[:, :]], auxA4, auxB4, 0)
                    store_u_h(u_all4, u4d, 0)
                elif c == NT - 1:
                    store_u_h(u_all4, u4d, 1)
                elif c == 4:
                    early_t0("ec4", lhs4, rhs4)
                elif c == 6:
                    early_t0_mid("ec4", lhs4, rhs4,
                                 [(sq4a[:, :], base3[0][:, :]),
                                  (sq4b[:, :], base3[1][:, :])], auxA4, auxB4)

            idxs3 = [idx3_t0]
            for c in range(NT):
                if c + 1 < NT:
                    idxs3.append(topk20(d3[c + 1]))
                gather_combine(idxs3[c], u3d, w_sb3[:, c, :], 256, c, x3[c], u_all3)
                transpose_into(x3[c][:, 0:P], base3[0], 0, c)
                transpose_into(x3[c][:, P:256], base3[1], 0, c)
                p1_unit(c, 0)
                prep4(c)
            # =====================================================
            # ec4
            # =====================================================
            x4 = [xp.tile([P, 256], BF16, tag=f"x3_{c}", bufs=2, name=f"x4_{c}")
                  for c in range(NT)]
            d4_t0, idx4_t0 = finish_t0("ec4", lhs4, rhs4,
                [(sq4a[:, :], base3[0][:, :]),
                 (sq4b[:, :], base3[1][:, :])], auxA4, auxB4)
            d4 = [d4_t0]
            d4 += dist_tiles(lhs4, rhs4, tiles=range(1, NT))
            base4 = [bp.tile([P, N], F32R, tag=f"base4_{i}", name=f"base4_{i}")
                     for i in range(2)]
            cat = [base3[0], base3[1], base4[0], base4[1]]
            p2t4 = mc.tile([P, 32], F32, tag="ptmp4", name="p2t4")

            def p2q_unit(j, q):
                # quarter-width x_t2 unit (256-wide keeps f32r matmuls at
                # 1 cycle/row; narrower runs at 1/4 rate): quarter q needs
                # only ec4 point tiles 2q, 2q+1, so all but the last
                # quarter overlap the gather phase
                qs = slice(q * 256, (q + 1) * 256)
                pst = pp.tile([P, 256], F32, tag="ps", name=f"pt2_{j}_{q}")
                for ki in range(4):
                    nc.tensor.matmul(pst[:], w5s[ki][:, cs(j)],
                                     cat[ki][:, qs],
                                     start=(ki == 0), stop=(ki == 3))
                nc.vector.tensor_reduce(
                    out=p2t4[:, 4 * j + q:4 * j + q + 1], in_=pst[:],
                    axis=AX.X, op=OP.max)

            fcolb = const.tile([P, 16], BF16)
            ps1 = pp.tile([1, 512], F32, tag="ps1", bufs=1, name="ps1")
            idxs4 = [topk20(d4[0], split_encode=True)]
            for c in range(NT):
                if c + 1 < NT:
                    idxs4.append(topk20(d4[c + 1]))
                gather_combine(idxs4[c], u4d, w_sb4[:, c, :], 256, c, x4[c], u_all4)
                transpose_into(x4[c][:, 0:P], base4[0], 0, c)
                transpose_into(x4[c][:, P:256], base4[1], 0, c)
                if c < 4:
                    # remaining x_t1 units (base1 stays live through ec4)
                    p1_unit(2 * c, 1)
                    p1_unit(2 * c + 1, 1)
                if c in (1, 2):
                    for j in range(4 * (c - 1), 4 * (c - 1) + 4):
                        p2q_unit(j, 0)
                elif c in (3, 4):
                    for j in range(4 * (c - 3), 4 * (c - 3) + 4):
                        p2q_unit(j, 1)
                elif c == 5:
                    for j in range(NT):
                        p2q_unit(j, 2)
                elif c == 6:
                    # PE p-state warm-up: the cost model halves matmul rate
                    # after any PE idle and needs 3us of continuous work to
                    # re-ramp. Keep PE busy through tile 7's gather window
                    # (where it would idle) so the tail's 32 x_t2 matmuls +
                    # classifier matmuls run at full rate, not half.
                    warm = pp.tile([P, 512], F32, tag="ps", name="warm")
                    for wi in range(16):
                        nc.tensor.matmul(warm[:], w5s[0][:, 0:P],
                                         base3[0][:, 0:512],
                                         start=True, stop=True)
                if c == 3:
                    # p1 is complete: fold its half of the classifier's
                    # first layer into the gather phase
                    p1pre = mc.tile([P, 8], F32, tag="ppre", name="p1pre")
                    nc.vector.tensor_reduce(
                        out=p1pre[:],
                        in_=p1tmp[:].rearrange("p (j h) -> p j h", h=2),
                        axis=AX.X, op=OP.max)
                    nc.vector.tensor_add(out=p1pre[:], in0=p1pre[:],
                                         in1=p1bias[:])
                    leaky(fcol[:, 0:8], p1pre[:])
                    nc.scalar.copy(out=fcolb[:, 0:8], in_=fcol[:, 0:8])
                    for j in range(8):
                        nc.tensor.matmul(ps1[:], fcolb[:, j:j + 1], l1ts[j][:],
                                         start=(j == 0), stop=False)
                    for j in range(12, 16):
                        nc.sync.dma_start(out=l1ts[j][:],
                                          in_=l1[j * P:(j + 1) * P, :])

            # =====================================================
            # x_t2 -> p2 tail (last quarter needs point tiles 6-7)
            # =====================================================
            for j in range(NT):
                p2q_unit(j, 3)
            p2pre = mc.tile([P, 8], F32, tag="ppre", name="p2pre")
            nc.vector.tensor_reduce(
                out=p2pre[:], in_=p2t4[:].rearrange("p (j q) -> p j q", q=4),
                axis=AX.X, op=OP.max)
            nc.vector.tensor_add(out=p2pre[:], in0=p2pre[:], in1=p2bias[:])
            leaky(fcol[:, 8:16], p2pre[:])

            # =====================================================
            # final MLP
            # =====================================================
            nc.scalar.copy(out=fcolb[:, 8:16], in_=fcol[:, 8:16])
            for j in range(8, 16):
                nc.tensor.matmul(ps1[:], fcolb[:, j:j + 1], l1ts[j][:],
                                 start=False, stop=False)
            nc.tensor.matmul(ps1[:], ones1[:1, :1], l1bs[:],
                             start=False, stop=True)
            f1sb = mc.tile([1, 512], F32, tag="f1pre", name="f1sb")
            leaky(f1sb[:], ps1[:])
            f2col = mc.tile([P, 4], BF16, tag="f2col", name="f2col")
            for j in range(4):
                pst = pp.tile([P, 1], F32, tag="ps", name=f"ptc1_{j}")
                nc.tensor.transpose(out=pst[:], in_=f1sb[:1, j * P:(j + 1) * P],
                                    identity=ident[:1, :1])
                nc.scalar.copy(out=f2col[:, j:j + 1], in_=pst[:])

            ps2 = pp.tile([1, 256], F32, tag="ps", name="ps2")
            for j in range(4):
                nc.tensor.matmul(ps2[:], f2col[:, j:j + 1], l2ts[j][:],
                                 start=(j == 0), stop=False)
            nc.tensor.matmul(ps2[:], ones1[:1, :1], l2bs[:],
                             start=False, stop=True)
            f2sb = mc.tile([1, 256], F32, tag="f2pre", name="f2sb")
            leaky(f2sb[:], ps2[:])
            f3col = mc.tile([P, 2], BF16, tag="f3col", name="f3col")
            for j in range(2):
                pst = pp.tile([P, 1], F32, tag="ps", name=f"ptc2_{j}")
                nc.tensor.transpose(out=pst[:], in_=f2sb[:1, j * P:(j + 1) * P],
                                    identity=ident[:1, :1])
                nc.scalar.copy(out=f3col[:, j:j + 1], in_=pst[:])

            ps3 = pp.tile([1, 40], F32, tag="ps", name="ps3")
            for j in range(2):
                nc.tensor.matmul(ps3[:], f3col[:, j:j + 1], l3ts[j][:],
                                 start=(j == 0), stop=False)
            nc.tensor.matmul(ps3[:], ones1[:1, :1], l3bs[:],
                             start=False, stop=True)
            osb = mc.tile([1, 40], F32, tag="osb", name="osb")
            nc.scalar.copy(out=osb[:], in_=ps3[:])
            nc.sync.dma_start(out=out_d[:], in_=osb[:])

    _split_excess_waits(nc)
    nc.finalize()
    return nc


def _split_excess_waits(nc, cap=1):
    """Walrus codegen rejects instructions with more than `cap` sem waits
    (matmul LDWEIGHTS allows only 1; most others take 2).
    Hoist the excess onto same-engine NOPs inserted just before."""
    for b in nc.m.functions[0].blocks:
        new = []
        changed = False
        for inst in b.instructions:
            cap = 1
            si = getattr(inst, "sync_info", None)
            if si is not None and si.on_wait is not None and len(si.on_wait) > cap:
                waits = list(si.on_wait)
                rest = waits[cap:]
                k = 0
                while rest:
                    chunk, rest = rest[:cap], rest[cap:]
                    nop = mybir.InstNoOp(name=f"{inst.name}-ws{k}", ins=[],
                                         outs=[])
                    nop.engine = inst.engine
                    nop.sync_info = mybir.SyncInfo(on_wait=chunk, on_update=[])
                    new.append(nop)
                    k += 1
                inst.sync_info = mybir.SyncInfo(on_wait=waits[:cap],
                                                on_update=list(si.on_update))
                changed = True
            new.append(inst)
        if changed:
            b.instructions = new


def _round_f32r(x):
    """Round f32 -> f32r bit pattern (13 explicit mantissa bits, RNE-ish)."""
    x = np.ascontiguousarray(x, np.float32)
    u = x.view(np.uint32).astype(np.uint64)
    r = ((u + (1 << 9)) & np.uint64(0xFFFFFC00)).astype(np.uint32)
    return r.view(np.float32)


def prep_weights(inp):
    """Host-side constant folding: BN scales/biases into weights, EdgeConv
    linear decomposition, transposes into lhsT/rhs layouts."""
    S = 1.0 / math.sqrt(1.0 + 1e-5)
    f = np.float32
    w = {}
    s1 = (inp["g1"] * S).astype(f)
    w["uW1"] = np.ascontiguousarray((s1[:, None] * inp["W1"][:, :3]).T, f)
    w["wW1"] = np.ascontiguousarray(
        (s1[:, None] * (inp["W1"][:, 3:] - inp["W1"][:, :3])).T, f)
    w["wb1"] = inp["b1"][None].astype(f)
    s2 = (inp["g2"] * S).astype(f)
    w["uW2"] = np.ascontiguousarray((s2[:, None] * inp["W2"][:, :64]).T, f)
    w["wW2"] = np.ascontiguousarray(
        (s2[:, None] * (inp["W2"][:, 64:] - inp["W2"][:, :64])).T, f)
    w["wb2"] = inp["b2"][None].astype(f)
    s3 = (inp["g3"] * S).astype(f)
    W3 = inp["W3"]
    Wa3 = W3[:, :256]; Wb3 = W3[:, 256:]
    Wa3e = Wa3[:, :128] + Wa3[:, 128:]
    Wb3e = Wb3[:, :128] + Wb3[:, 128:]
    w["uW3"] = np.ascontiguousarray((s3[:, None] * Wa3e).T, f)
    w["wW3"] = np.ascontiguousarray((s3[:, None] * (Wb3e - Wa3e)).T, f)
    w["wb3"] = inp["b3"][None].astype(f)
    s4 = (inp["g4"] * S).astype(f)
    w["uW4"] = np.ascontiguousarray((s4[:, None] * inp["W4"][:, :256]).T, f)
    w["wW4"] = np.ascontiguousarray(
        (s4[:, None] * (inp["W4"][:, 256:] - inp["W4"][:, :256])).T, f)
    w["wb4"] = inp["b4"][None].astype(f)
    s2m = (inp["g2m"] * S).astype(f)
    w["w2m"] = np.ascontiguousarray((s2m[:, None] * inp["W2m"]).T, f)
    w["w2mb"] = inp["b2m"][None].astype(f)
    s5 = (inp["g5"] * S).astype(f)
    w["w5"] = np.ascontiguousarray((s5[:, None] * inp["W5"]).T, f)
    w["w5b"] = inp["b5"][None].astype(f)
    s6 = (inp["g6"] * S).astype(f)
    w["l1"] = np.ascontiguousarray((s6[:, None] * inp["L1"]).T, f)
    w["l1b"] = inp["b6"][None].astype(f)
    s7 = (inp["g7"] * S).astype(f)
    w["l2"] = np.ascontiguousarray((s7[:, None] * inp["L2"]).T, f)
    w["l2b"] = (s7 * inp["bL2"] + inp["b7"])[None].astype(f)
    w["l3"] = np.ascontiguousarray(inp["L3"].T, f)
    w["l3b"] = inp["bL3"][None].astype(f)
    import ml_dtypes
    out = {}
    for k, v in w.items():
        if k in ("l1", "l2", "l3"):
            out[k] = np.ascontiguousarray(v.astype(ml_dtypes.bfloat16))
        else:
            out[k] = _round_f32r(v)
    return out


_NC_CACHE = None


def get_nc():
    global _NC_CACHE
    if _NC_CACHE is None:
        _NC_CACHE = build_nc()
    return _NC_CACHE


def run(inputs, trace=False):
    nc = get_nc()
    w = prep_weights(inputs)
    x = np.asarray(inputs["x"], np.float32)
    in_maps = [{"xs": _round_f32r(x[i]), **w} for i in range(8)]
    res = run_bass_kernel_spmd(nc, in_maps, core_ids=list(range(8)), trace=trace)
    out = np.concatenate([res.results[i]["out"] for i in range(8)], axis=0)
    return out, res


def kernel(**inputs) -> np.ndarray:
    out, _ = run(inputs)
    return out.astype(np.float32)



# revision 77
# speedup vs baseline: 1.0000x; 1.0000x over previous
"""DGCNN classifier forward as a distributed Bass kernel on 8 TRN2 NeuronCores.

Sharding: pure data-parallel, one sample per core (B=8).

Per-core algorithm:
  - EdgeConv(x, W) decomposes: y[n,k] = Wa@x_j + (Wb-Wa)@x_i, so with
    u = x@Wa', w = x@(Wb-Wa)' + b (BN scale/bias folded host-side, scale>0),
    out[n] = leaky(max_k u[idx[n,k]] + w[n]); leaky(x) = max(0.2x, x) is a
    single DVE STT pass.
  - KNN: neg half-dist = A.T@B with A = [x; 1; -xx/2], B = [x; -xx/2-c; 1]
    on PE (c = 2^-40 keeps the self column away from denormals); the column
    id is planted in the low 10 mantissa bits (DVE STT with an iota tile).
    Topk is two-level: max8 per 128-col block (8 cheap passes) then 3x max8
    + 2x match_replace merge over the 64 candidates -- exact unless a block
    holds >8 of a point's true top-20 (68/32768 points here, 3.4e-3 rel-err
    contribution). Rank 1 is always self; ranks 2..20 are gathered via
    per-neighbor SWDGE indirect DMAs (the only SWDGE form walrus supports;
    500ns Pool floor each puts Pool at ~85% busy -- THE bottleneck), maxed
    by a progressive bf16 tensor-max tree that consumes gather rounds as
    they land.
  - Software pipelining hides everything else under the gather stream:
    per-tile u/w matmuls, squares, aux builds and u-table stores of EC i+1
    are emitted inside EC i's gather loop; cols 0-895 of the next EC's
    first dist+topk run early so only a 128-col sliver + merge sit on each
    boundary; x_t1/x_t2 pool units interleave as their point-tiles land
    (256-wide quarters keep f32r matmuls at full rate); p1's half of the
    classifier accumulates mid-loop. Warm-up matmuls keep the PE p-state
    ramped through the start-of-kernel load window and the last gather
    window (idle resets matmuls to half rate). Const-setup and aux copies
    run on DVE, keeping the ACT queue free for the sq/dist copies that
    gate each topk.
  - nf1 = [xt1,xt1] duplication: distances are 2x those of xt1 (same topk
    order) and the 512-wide conv folds to 256-wide (W halves summed).
  - x_t1/x_t2 only feed max-pools: computed channel-major, leaky after max.

Cost-model time: 347923 ns/core (baseline 404786), rel err 1.21e-2.
"""

import math
import numpy as np

import concourse.bass as bass
import concourse.mybir as mybir
import concourse.tile as tile
from concourse.bass_utils import run_bass_kernel_spmd
from concourse.masks import make_identity

F32 = mybir.dt.float32
F32R = mybir.dt.float32r
U32 = mybir.dt.uint32
BF16 = mybir.dt.bfloat16
AX = mybir.AxisListType
OP = mybir.AluOpType
ACTF = mybir.ActivationFunctionType

P = 128
N = 1024
NT = N // P          # 8 point tiles
KNN = 20
NEG = -3.0e38


def build_nc():
    nc = bass.Bass()

    def _in(name, shape):
        # weights are pre-rounded to f32r bit patterns on the host
        return nc.declare_dram_parameter(name, shape, F32R, isOutput=False)

    xs = nc.declare_dram_parameter("xs", [3, N], F32R, isOutput=False)
    uW1 = _in("uW1", [3, 64]); wW1 = _in("wW1", [3, 64]); wb1 = _in("wb1", [1, 64])
    uW2 = _in("uW2", [64, 64]); wW2 = _in("wW2", [64, 64]); wb2 = _in("wb2", [1, 64])
    uW3 = _in("uW3", [128, 256]); wW3 = _in("wW3", [128, 256]); wb3 = _in("wb3", [1, 256])
    uW4 = _in("uW4", [256, 256]); wW4 = _in("wW4", [256, 256]); wb4 = _in("wb4", [1, 256])
    w2m = _in("w2m", [128, N]); w2mb = _in("w2mb", [1, N])
    w5 = _in("w5", [512, N]); w5b = _in("w5b", [1, N])
    l1 = nc.declare_dram_parameter("l1", [2048, 512], BF16, isOutput=False)
    l1b = _in("l1b", [1, 512])
    l2 = nc.declare_dram_parameter("l2", [512, 256], BF16, isOutput=False)
    l2b = _in("l2b", [1, 256])
    l3 = nc.declare_dram_parameter("l3", [256, 40], BF16, isOutput=False)
    l3b = _in("l3b", [1, 40])
    out_d = nc.declare_dram_parameter("out", [1, 40], F32, isOutput=True)

    with tile.TileContext(nc) as tc:
        with (
            tc.tile_pool(name="const", bufs=1) as const,
            tc.tile_pool(name="wpool", bufs=1) as wp,
            tc.tile_pool(name="base", bufs=1) as bp,
            tc.tile_pool(name="sq", bufs=1) as sqp,
            tc.tile_pool(name="dist", bufs=6) as dp,
            tc.tile_pool(name="vals", bufs=6) as vp,
            tc.tile_pool(name="idx", bufs=6) as ip,
            tc.tile_pool(name="uw", bufs=1) as uwp,
            tc.tile_pool(name="gat", bufs=3) as gp,
            tc.tile_pool(name="xout", bufs=1) as xp,
            tc.tile_pool(name="mlp", bufs=8) as mp,
            tc.tile_pool(name="misc", bufs=1) as mc,
            tc.tile_pool(name="lk", bufs=1) as lkp,
            tc.tile_pool(name="ps", bufs=7, space="PSUM") as pp,
            tc.tile_pool(name="dram", bufs=1, space="DRAM") as dr,
        ):
            def leaky(out, in_):
                # leaky(x) = max(0.2x, x): a single DVE STT pass. PSUM
                # sources may only be read once per instruction, so those
                # fall back to ACT relu(-0.8x) + DVE add.
                if in_.space == bass.MemorySpace.PSUM:
                    shp = list(in_.shape)
                    fs = in_.free_size()
                    rt = lkp.tile([1, 512], in_.dtype, tag="lk1f", bufs=1,
                                  name=f"lk_{nc.next_id()}")
                    rv = rt[:shp[0], :fs]
                    nc.scalar.activation(out=rv, in_=in_, func=ACTF.Relu,
                                         scale=-0.8)
                    nc.vector.tensor_add(out=out, in0=in_, in1=rv)
                else:
                    nc.vector.scalar_tensor_tensor(
                        out=out, in0=in_, scalar=0.2, in1=in_,
                        op0=OP.mult, op1=OP.max)

            ident = const.tile([P, P], F32)
            make_identity(nc, ident[:])
            identb = const.tile([P, P], BF16)
            nc.vector.tensor_copy(out=identb[:], in_=ident[:])
            # memset can't emit f32r: init an f32 twin, ACT-copy (casts)
            cscr = const.tile([P, 4], F32)    # cols: [1, 0, -0.5, -2^-40]
            nc.vector.memset(cscr[:, 0:1], 1.0)
            nc.vector.memset(cscr[:, 1:2], 0.0)
            nc.vector.memset(cscr[:, 2:3], -0.5)
            nc.vector.memset(cscr[:, 3:4], -(2.0 ** -40))
            # index-in-value topk constants: distances are quantized to 13
            # mantissa bits and the column id is planted in the low 10 bits,
            # so max8 winners carry their own indices (no max_index passes).
            iotac = const.tile([P, N], U32)
            nc.gpsimd.iota(iotac[:], pattern=[[1, N]], base=0,
                           channel_multiplier=0)
            maskcol = const.tile([P, 1], U32)
            nc.gpsimd.iota(maskcol[:], pattern=[[0, 1]], base=0xFFFFFC00,
                           channel_multiplier=0)
            andcol = const.tile([P, 1], U32)
            nc.gpsimd.iota(andcol[:], pattern=[[0, 1]], base=0x3FF,
                           channel_multiplier=0)
            ones1f = const.tile([1, N], F32)
            nc.vector.memset(ones1f[:], 1.0)
            ones1 = const.tile([1, N], F32R)
            nc.vector.tensor_copy(out=ones1[:], in_=ones1f[:])
            znA = const.tile([P, 2], F32R)    # cols [0,-1]
            nc.vector.tensor_copy(out=znA[:], in_=cscr[:, 1:3])
            znB = const.tile([P, 2], F32R)    # cols [-1,0]
            nc.vector.tensor_copy(out=znB[:, 0:1], in_=cscr[:, 2:3])
            nc.vector.tensor_copy(out=znB[:, 1:2], in_=cscr[:, 1:2])
            oA = const.tile([1, 2], F32R)     # [1,0]
            nc.vector.tensor_copy(out=oA[:], in_=cscr[:1, 0:2])
            # oB = [-2^-40, 1]: shifts every distance by -2^-40 so the self
            # column (dist 0) encodes as a normal float, never a denormal
            oB = const.tile([1, 2], F32R)
            nc.vector.tensor_copy(out=oB[:, 0:1], in_=cscr[:1, 3:4])
            nc.vector.tensor_copy(out=oB[:, 1:2], in_=cscr[:1, 0:1])

            # ramp warm-up: run the PE during the x-load window so the
            # first aux/dist matmuls start at full p-state, not cold
            wrm0 = pp.tile([P, P], F32, tag="ps", name="wrm0")
            for _ in range(5):
                nc.tensor.matmul(wrm0[:], ident[:, 0:P], ident[:],
                                 start=True, stop=True)

            # ---------- weights ----------
            def _load(dram, shape, eng=None):
                t = wp.tile(shape, F32R, tag=dram.name, name=f"{dram.name}_s")
                (eng or nc.gpsimd).dma_start(out=t[:], in_=dram[:])
                return t

            # the point cloud loads FIRST: it heads the ramp's serial chain
            # (sq -> aux -> dist -> topk); ec1's weights only feed psu/psw
            # which run later
            base0 = bp.tile([3, N], F32R, tag="base0")
            nc.sync.dma_start(out=base0[:3, 0:512], in_=xs[:, 0:512])
            nc.sync.dma_start(out=base0[:3, 512:], in_=xs[:, 512:])
            uW1s = _load(uW1, [3, 64], nc.sync); wW1s = _load(wW1, [3, 64], nc.sync)
            wb1s = _load(wb1, [1, 64], nc.sync)
            u1d = dr.tile([N, 64], BF16)
            u2d = dr.tile([N, 64], BF16)
            u3d = dr.tile([N, 256], BF16)
            u4d = dr.tile([N, 256], BF16)

            # ---------- helpers ----------
            def cs(c):
                return slice(c * P, (c + 1) * P)

            def dist_half(lhs_ktiles, rhs_ktiles, d, c, h):
                hs = slice(h * 512, (h + 1) * 512)
                psd = pp.tile([P, 512], F32, tag="ps",
                              name=f"psd_{nc.next_id()}")
                nk = len(lhs_ktiles)
                for ki, (lf, rf) in enumerate(zip(lhs_ktiles, rhs_ktiles)):
                    nc.tensor.matmul(psd[:], lf(c), rf(hs),
                                     start=(ki == 0), stop=(ki == nk - 1))
                nc.scalar.copy(out=d[:, hs], in_=psd[:])

            def dist_tiles(lhs_ktiles, rhs_ktiles, tiles=None):
                out = []
                for c in (range(NT) if tiles is None else tiles):
                    d = dp.tile([P, N], F32, tag="dist", name=f"d_{nc.next_id()}")
                    dist_half(lhs_ktiles, rhs_ktiles, d, c, 0)
                    dist_half(lhs_ktiles, rhs_ktiles, d, c, 1)
                    out.append(d)
                return out

            def topk_half(d, v64, h):
                # encode: plant the column id in the low 10 mantissa bits
                # (DVE only: TensorScalarPtr is illegal on the Pool engine),
                # then max8 each 128-col block (8 cheap passes over 128
                # elems instead of 3+2 passes over 1024)
                hs = slice(h * 512, (h + 1) * 512)
                nc.vector.scalar_tensor_tensor(
                    out=d[:, hs].bitcast(U32), in0=d[:, hs].bitcast(U32),
                    scalar=maskcol[:], in1=iotac[:, hs],
                    op0=OP.bitwise_and, op1=OP.bitwise_or)
                for blk in range(4):
                    b = 4 * h + blk
                    nc.vector.max(out=v64[:, b * 8:(b + 1) * 8],
                                  in_=d[:, b * 128:(b + 1) * 128])

            def topk_merge(v64):
                # top-20 of the 64 block candidates: 3x max8 + 2x
                # match_replace on [P, 64]. Exact unless a block holds >8
                # of a point's true top-20 (measured: 68/32768 points on
                # this dataset, end-to-end rel err contribution 3.4e-3).
                # Rank 1 is always the self column (its shifted distance
                # -2^-40 beats every real distance).
                v = vp.tile([P, 24], F32, tag="vals", name=f"v_{nc.next_id()}")
                idx24 = ip.tile([P, 24], U32, tag="idx", name=f"idx_{nc.next_id()}")
                for r in range(3):
                    rs = slice(r * 8, (r + 1) * 8)
                    nc.vector.max(out=v[:, rs], in_=v64[:])
                    if r < 2:
                        nc.vector.match_replace(
                            out=v64[:], in_to_replace=v[:, rs],
                            in_values=v64[:], imm_value=NEG)
                    # per-round decode so the first gathers can launch after
                    # round 1 instead of waiting for the whole topk
                    nc.vector.tensor_scalar(out=idx24[:, rs],
                                            in0=v[:, rs].bitcast(U32),
                                            scalar1=andcol[:], scalar2=None,
                                            op0=OP.bitwise_and)
                return idx24

            def topk20(d, split_encode=False):
                v64 = vp.tile([P, 64], F32, tag="cand", bufs=4,
                              name=f"c_{nc.next_id()}")
                topk_half(d, v64, 0)
                topk_half(d, v64, 1)
                return topk_merge(v64)

            def gather_combine(idx24, ud, w_ap, C, c, xtile, uall):
                """xtile = leaky(max_k u[idx] + w).

                Rank-1 is always self, whose u-row already sits in this
                partition of u_all — ACT-copy it into slot 0. Ranks 2..20
                come via 19 per-column indirect gathers (one offset per
                partition — the only SWDGE form walrus descriptor-gen
                supports; multi-offset APs scramble), contiguous [P, C]
                dest slices of one wide tile."""
                big = gp.tile([P, KNN * C], BF16, tag="gat", bufs=3,
                              name=f"g_{nc.next_id()}")
                nc.scalar.copy(out=big[:, 0:C], in_=uall[:, c, :])
                for t in range(1, KNN):
                    nc.gpsimd.indirect_dma_start(
                        out=big[:, t * C:(t + 1) * C], out_offset=None,
                        in_=ud[:],
                        in_offset=bass.IndirectOffsetOnAxis(
                            ap=idx24[:, t:t + 1], axis=0),
                    )
                # progressive bf16 TT-max tree (2-byte packed -> DVE 2x
                # mode): ordered so everything except one C-wide fold is
                # done before the LAST gather chunk lands -- only
                # max+add+leaky remain on the post-gather critical path
                nc.vector.tensor_max(out=big[:, :4 * C], in0=big[:, :4 * C],
                                     in1=big[:, 4 * C:8 * C])
                nc.vector.tensor_max(out=big[:, :4 * C], in0=big[:, :4 * C],
                                     in1=big[:, 8 * C:12 * C])
                nc.vector.tensor_max(out=big[:, :4 * C], in0=big[:, :4 * C],
                                     in1=big[:, 12 * C:16 * C])
                nc.vector.tensor_max(out=big[:, :2 * C], in0=big[:, :2 * C],
                                     in1=big[:, 16 * C:18 * C])
                nc.vector.tensor_max(out=big[:, :2 * C], in0=big[:, :2 * C],
                                     in1=big[:, 2 * C:4 * C])
                nc.vector.tensor_max(out=big[:, :C], in0=big[:, :C],
                                     in1=big[:, C:2 * C])
                nc.vector.tensor_max(out=big[:, :C], in0=big[:, :C],
                                     in1=big[:, 18 * C:19 * C])
                nc.vector.tensor_max(out=big[:, :C], in0=big[:, :C],
                                     in1=big[:, 19 * C:20 * C])
                m = uwp.tile([P, C], BF16, tag="m", bufs=3,
                             name=f"m_{nc.next_id()}")
                nc.vector.tensor_add(out=m[:], in0=big[:, :C], in1=w_ap)
                leaky(xtile[:], m[:])

            def build_aux(sq_aps, auxA, auxB):
                """auxA = [ones; -xx], auxB = [-xx; ones] via PE rank updates."""
                for h in range(2):
                    build_aux_h(sq_aps, auxA, auxB, h)

            def build_aux_cols(sq_aps, auxA, auxB, lo, hi):
                # auxB first: the next dist half's rhs needs it immediately,
                # while auxA's lhs col-slice is only read per point-tile
                for aux, orow, zn in ((auxB, oB, znB), (auxA, oA, znA)):
                    psa = pp.tile([2, hi - lo], F32, tag="ps",
                                  name=f"psa_{nc.next_id()}")
                    nc.tensor.matmul(psa[:], orow[:1, :2], ones1[:1, lo:hi],
                                     start=True, stop=False)
                    for i, sqa in enumerate(sq_aps):
                        kp = sqa.shape[0]
                        nc.tensor.matmul(psa[:], zn[:kp, :2], sqa[:, lo:hi],
                                         start=False,
                                         stop=(i == len(sq_aps) - 1))
                    nc.vector.tensor_copy(out=aux[0:2, lo:hi], in_=psa[:])

            def build_aux_h(sq_aps, auxA, auxB, h):
                build_aux_cols(sq_aps, auxA, auxB, h * 512, (h + 1) * 512)

            def transpose_into(src_ap, dst_tile, dst_row0, c):
                w = src_ap.shape[1]
                pst = pp.tile([w, P], src_ap.dtype, tag="ps",
                              name=f"pst_{nc.next_id()}")
                idt = identb if src_ap.dtype == BF16 else ident
                nc.tensor.transpose(out=pst[:], in_=src_ap, identity=idt[:])
                nc.scalar.copy(out=dst_tile[dst_row0:dst_row0 + w, cs(c)],
                               in_=pst[:])

            def store_u_h(u_all, ud, h):
                # half stores so each can launch after its 4 psu copies,
                # pulling the store off the first-gather critical path
                half = N // 2
                nc.sync.dma_start(
                    out=ud[h * half:(h + 1) * half].rearrange(
                        "(c p) f -> p c f", p=P),
                    in_=u_all[:, h * 4:(h + 1) * 4, :])

            def store_u(u_all, ud):
                store_u_h(u_all, ud, 0)
                store_u_h(u_all, ud, 1)

            # =====================================================
            # ec1
            # =====================================================
            auxA0 = bp.tile([2, N], F32R, tag="auxA", bufs=2)
            auxB0 = bp.tile([2, N], F32R, tag="auxB", bufs=2)
            sq0 = sqp.tile([3, N], F32R, tag="sq", bufs=2, name="sq0")
            nc.scalar.square(out=sq0[:, 0:512], in_=base0[:3, 0:512])
            nc.scalar.square(out=sq0[:, 512:], in_=base0[:3, 512:])
            build_aux([sq0[:3, :]], auxA0, auxB0)

            x1 = [xp.tile([P, 64], BF16, tag=f"x1_{c}", bufs=2, name=f"x1_{c}")
                  for c in range(NT)]
            u_all1 = uwp.tile([P, NT, 64], BF16, tag="uall64", bufs=2, name="ua1")
            for c in range(NT):
                psu = pp.tile([P, 64], F32, tag="ps", name=f"psu1_{c}")
                nc.tensor.matmul(psu[:], base0[:3, cs(c)], uW1s[:])
                nc.scalar.copy(out=u_all1[:, c, :], in_=psu[:])
                if c in (3, NT - 1):
                    store_u_h(u_all1, u1d, 0 if c == 3 else 1)
            lhs1 = [lambda c: base0[:3, cs(c)], lambda c: auxA0[0:2, cs(c)]]
            rhs1 = [lambda hs: base0[:3, hs], lambda hs: auxB0[0:2, hs]]
            d1 = dist_tiles(lhs1, rhs1, tiles=[0])
            w_sb1 = uwp.tile([P, NT, 64], BF16, tag="wall64", bufs=2,
                             name="wsb1")
            for c in range(NT):
                psw = pp.tile([P, 64], F32, tag="ps", name=f"psw1_{c}")
                nc.tensor.matmul(psw[:], base0[:3, cs(c)], wW1s[:],
                                 start=True, stop=False)
                nc.tensor.matmul(psw[:], ones1[:1, cs(c)], wb1s[:],
                                 start=False, stop=True)
                nc.scalar.copy(out=w_sb1[:, c, :], in_=psw[:])
            d1 += dist_tiles(lhs1, rhs1, tiles=range(1, NT))
            base1 = bp.tile([P, N], F32R, tag="base1")
            # bulk weight loads for ec2..mlp on the SP queue (off Pool; SP
            # drains all of this during ec1's gather phase)
            uW2s = _load(uW2, [64, 64], nc.sync); wW2s = _load(wW2, [64, 64], nc.sync)
            wb2s = _load(wb2, [1, 64], nc.sync)
            uW3s = _load(uW3, [128, 256], nc.sync); wW3s = _load(wW3, [128, 256], nc.sync)
            wb3s = _load(wb3, [1, 256], nc.sync); wb4s = _load(wb4, [1, 256], nc.sync)
            uW4s = [wp.tile([P, 256], F32R, tag=f"uW4_{i}", name=f"uW4s_{i}")
                    for i in range(2)]
            wW4s = [wp.tile([P, 256], F32R, tag=f"wW4_{i}", name=f"wW4s_{i}")
                    for i in range(2)]
            for i in range(2):
                nc.sync.dma_start(out=uW4s[i][:], in_=uW4[i * P:(i + 1) * P, :])
                nc.sync.dma_start(out=wW4s[i][:], in_=wW4[i * P:(i + 1) * P, :])
            w2ms = _load(w2m, [128, N], nc.sync)
            p1bias = wp.tile([P, NT], F32R, tag="p1bias", name="p1bias")
            nc.sync.dma_start(
                out=p1bias[:], in_=w2mb[:].rearrange("o (j p) -> (o p) j", p=P))
            w5s = [wp.tile([P, N], F32R, tag=f"w5_{i}", name=f"w5s_{i}")
                   for i in range(4)]
            for i in range(4):
                nc.sync.dma_start(out=w5s[i][:], in_=w5[i * P:(i + 1) * P, :])
            p2bias = wp.tile([P, NT], F32R, tag="p2bias", name="p2bias")
            nc.sync.dma_start(
                out=p2bias[:], in_=w5b[:].rearrange("o (j p) -> (o p) j", p=P))
            l1bs = _load(l1b, [1, 512], nc.sync); l2bs = _load(l2b, [1, 256], nc.sync)
            l3bs = _load(l3b, [1, 40], nc.sync)
            # prefetch the classifier weights on the SP queue too (it drains
            # all of this during ec1, well before its next dependent op)
            l1ts = [mp.tile([P, 512], BF16, tag=f"l1_{j % 12}", bufs=1,
                            name=f"l1t_{j}") for j in range(16)]
            for j in range(12):
                nc.sync.dma_start(out=l1ts[j][:], in_=l1[j * P:(j + 1) * P, :])
            l2ts = [mp.tile([P, 256], BF16, tag=f"l2_{j}", bufs=1, name=f"l2t_{j}")
                    for j in range(4)]
            for j in range(4):
                nc.sync.dma_start(out=l2ts[j][:], in_=l2[j * P:(j + 1) * P, :])
            l3ts = [mp.tile([P, 40], BF16, tag=f"l3_{j}", bufs=1, name=f"l3t_{j}")
                    for j in range(2)]
            for j in range(2):
                nc.sync.dma_start(out=l3ts[j][:], in_=l3[j * P:(j + 1) * P, :])

            # ec2 prep, hoisted into ec1's gather loop: per-tile u/w matmuls
            # as soon as tile c of base1 is transposed, sq/aux/u-store per
            # half at c==3/7 — so the ec1->ec2 boundary only has to run
            # dist+topk of ec2's first tile before its gathers start.
            auxA2 = bp.tile([2, N], F32R, tag="auxA", bufs=2)
            auxB2 = bp.tile([2, N], F32R, tag="auxB", bufs=2)
            sq2 = sqp.tile([64, N], F32R, tag="sq", bufs=2, name="sq2")
            u_all2 = uwp.tile([P, NT, 64], BF16, tag="uall64", bufs=2, name="ua2")
            w_sb2 = uwp.tile([P, NT, 64], BF16, tag="wall64", bufs=2,
                             name="wsb2")
            lhs2 = [lambda c: base1[:64, cs(c)], lambda c: auxA2[0:2, cs(c)]]
            rhs2 = [lambda hs: base1[:64, hs], lambda hs: auxB2[0:2, hs]]
            early = {}

            def dist_cols(lhs_ktiles, rhs_ktiles, d, c, lo, hi):
                # dist over a column sub-range [lo, hi) of tile c
                psd = pp.tile([P, hi - lo], F32, tag="ps",
                              name=f"psd_{nc.next_id()}")
                nk = len(lhs_ktiles)
                for ki, (lf, rf) in enumerate(zip(lhs_ktiles, rhs_ktiles)):
                    nc.tensor.matmul(psd[:], lf(c), rf(slice(lo, hi)),
                                     start=(ki == 0), stop=(ki == nk - 1))
                nc.scalar.copy(out=d[:, lo:hi], in_=psd[:])

            def topk_blocks(d, v64, blo, bhi):
                # encode + block max8 for 128-col blocks [blo, bhi)
                nc.vector.scalar_tensor_tensor(
                    out=d[:, blo * 128:bhi * 128].bitcast(U32),
                    in0=d[:, blo * 128:bhi * 128].bitcast(U32),
                    scalar=maskcol[:], in1=iotac[:, blo * 128:bhi * 128],
                    op0=OP.bitwise_and, op1=OP.bitwise_or)
                for b in range(blo, bhi):
                    nc.vector.max(out=v64[:, b * 8:(b + 1) * 8],
                                  in_=d[:, b * 128:(b + 1) * 128])

            def early_t0(key, lhs, rhs):
                # cols 0-511 of the next EC's first dist+topk, emitted while
                # the current EC's gathers still stream (needs aux h0 +
                # output tiles 0-3 only)
                d = dp.tile([P, N], F32, tag="dist", name=f"d_{nc.next_id()}")
                v64 = vp.tile([P, 64], F32, tag="cand", bufs=4,
                              name=f"c_{nc.next_id()}")
                dist_half(lhs, rhs, d, 0, 0)
                topk_half(d, v64, 0)
                early[key] = (d, v64)

            def early_t0_mid(key, lhs, rhs, sqs, auxA, auxB):
                # cols 512-895: aux/dist/encode over everything but the
                # last 128-col sliver, emitted once output tiles 4-6 exist
                for sqa, base_ap in sqs:
                    nc.scalar.square(out=sqa[:, 512:896], in_=base_ap[:, 512:896])
                build_aux_cols([s for s, _ in sqs], auxA, auxB, 512, 896)
                d, v64 = early[key]
                dist_cols(lhs, rhs, d, 0, 512, 896)
                topk_blocks(d, v64, 4, 7)

            def finish_t0(key, lhs, rhs, sqs, auxA, auxB):
                # the last 128-col sliver + merge: the only topk work left
                # on the boundary critical path
                for sqa, base_ap in sqs:
                    nc.scalar.square(out=sqa[:, 896:], in_=base_ap[:, 896:])
                build_aux_cols([s for s, _ in sqs], auxA, auxB, 896, 1024)
                d, v64 = early[key]
                dist_cols(lhs, rhs, d, 0, 896, 1024)
                topk_blocks(d, v64, 7, 8)
                return d, topk_merge(v64)

            def prep2(c):
                # at the last tile, sq/aux jump ahead of psu/psw on the
                # ACT queue: they gate the next EC's first dist+topk half
                psu = pp.tile([P, 64], F32, tag="ps", name=f"psu2_{c}")
                nc.tensor.matmul(psu[:], base1[:64, cs(c)], uW2s[:])
                nc.scalar.copy(out=u_all2[:, c, :], in_=psu[:])
                psw = pp.tile([P, 64], F32, tag="ps", name=f"psw2_{c}")
                nc.tensor.matmul(psw[:], base1[:64, cs(c)], wW2s[:],
                                 start=True, stop=False)
                nc.tensor.matmul(psw[:], ones1[:1, cs(c)], wb2s[:],
                                 start=False, stop=True)
                nc.scalar.copy(out=w_sb2[:, c, :], in_=psw[:])
                if c == 3:
                    nc.scalar.square(out=sq2[:, 0:512], in_=base1[:64, 0:512])
                    build_aux_h([sq2[:64, :]], auxA2, auxB2, 0)
                    store_u_h(u_all2, u2d, 0)
                elif c == NT - 1:
                    store_u_h(u_all2, u2d, 1)
                elif c == 4:
                    early_t0("ec2", lhs2, rhs2)
                elif c == 6:
                    early_t0_mid("ec2", lhs2, rhs2,
                                 [(sq2[:64, :], base1[:64, :])], auxA2, auxB2)

            idxs1 = [topk20(d1[0], split_encode=True)]
            for c in range(NT):
                if c + 1 < NT:
                    idxs1.append(topk20(d1[c + 1]))
                gather_combine(idxs1[c], u1d, w_sb1[:, c, :], 64, c, x1[c], u_all1)
                transpose_into(x1[c][:, :64], base1, 0, c)
                prep2(c)

            # =====================================================
            # ec2
            # =====================================================
            x2 = [xp.tile([P, 64], BF16, tag=f"x1_{c}", bufs=2, name=f"x2_{c}")
                  for c in range(NT)]
            d2_t0, idx2_t0 = finish_t0("ec2", lhs2, rhs2,
                [(sq2[:64, :], base1[:64, :])], auxA2, auxB2)
            d2 = [d2_t0]
            d2 += dist_tiles(lhs2, rhs2, tiles=range(1, NT))

            # ec3 prep (hoisted into ec2's loop); ec3 runs on the 128-dim
            # xt1 = [x1; x2], so tile c is ready after ec2's transpose c
            auxA3 = bp.tile([2, N], F32R, tag="auxA", bufs=2)
            auxB3 = bp.tile([2, N], F32R, tag="auxB", bufs=2)
            sq3 = sqp.tile([P, N], F32R, tag="sq", bufs=2, name="sq3")
            u_all3 = uwp.tile([P, NT, 256], BF16, tag="uall256", bufs=2, name="ua3")
            w_sb3 = uwp.tile([P, NT, 256], BF16, tag="wall256", bufs=2,
                             name="wsb3")
            lhs3 = [lambda c: base1[:, cs(c)], lambda c: auxA3[0:2, cs(c)]]
            rhs3 = [lambda hs: base1[:, hs], lambda hs: auxB3[0:2, hs]]

            def prep3(c):
                psu = pp.tile([P, 256], F32, tag="ps", name=f"psu3_{c}")
                nc.tensor.matmul(psu[:], base1[:, cs(c)], uW3s[:])
                nc.scalar.copy(out=u_all3[:, c, :], in_=psu[:])
                psw = pp.tile([P, 256], F32, tag="ps", name=f"psw3_{c}")
                nc.tensor.matmul(psw[:], base1[:, cs(c)], wW3s[:],
                                 start=True, stop=False)
                nc.tensor.matmul(psw[:], ones1[:1, cs(c)], wb3s[:],
                                 start=False, stop=True)
                nc.scalar.copy(out=w_sb3[:, c, :], in_=psw[:])
                if c == 3:
                    nc.scalar.square(out=sq3[:, 0:512], in_=base1[:, 0:512])
                    build_aux_h([sq3[:, :]], auxA3, auxB3, 0)
                    store_u_h(u_all3, u3d, 0)
                elif c == NT - 1:
                    store_u_h(u_all3, u3d, 1)
                elif c == 4:
                    early_t0("ec3", lhs3, rhs3)
                elif c == 6:
                    early_t0_mid("ec3", lhs3, rhs3,
                                 [(sq3[:, :], base1[:, :])], auxA3, auxB3)

            idxs2 = [idx2_t0]
            for c in range(NT):
                if c + 1 < NT:
                    idxs2.append(topk20(d2[c + 1]))
                gather_combine(idxs2[c], u2d, w_sb2[:, c, :], 64, c, x2[c], u_all2)
                transpose_into(x2[c][:, :64], base1, 64, c)
                prep3(c)

            # =====================================================
            # ec3 (on 128-dim xt1)
            # =====================================================
            x3 = [xp.tile([P, 256], BF16, tag=f"x3_{c}", bufs=2, name=f"x3_{c}")
                  for c in range(NT)]
            # x_t1 -> p1 units (channel-major, leaky after max), interleaved
            # into ec3's gather phase where PE/DVE have slack
            fcol = const.tile([P, 16], F32R)
            p1tmp = mc.tile([P, 16], F32, tag="ptmp", name="p1tmp")

            def p1_unit(j, h):
                hs = slice(h * 512, (h + 1) * 512)
                pst = pp.tile([P, 512], F32, tag="ps", name=f"pt1_{j}_{h}")
                nc.tensor.matmul(pst[:], w2ms[:, cs(j)], base1[:, hs])
                nc.vector.tensor_reduce(
                    out=p1tmp[:, 2 * j + h:2 * j + h + 1], in_=pst[:],
                    axis=AX.X, op=OP.max)

            d3_t0, idx3_t0 = finish_t0("ec3", lhs3, rhs3,
                [(sq3[:, :], base1[:, :])], auxA3, auxB3)
            d3 = [d3_t0]
            d3 += dist_tiles(lhs3, rhs3, tiles=range(1, NT))
            base3 = [bp.tile([P, N], F32R, tag=f"base3_{i}", name=f"base3_{i}")
                     for i in range(2)]

            # ec4 prep (hoisted into ec3's gather loop)
            auxA4 = bp.tile([2, N], F32R, tag="auxA", bufs=2)
            auxB4 = bp.tile([2, N], F32R, tag="auxB", bufs=2)
            sq4a = sqp.tile([P, N], F32R, tag="sq", bufs=2, name="sq4a")
            sq4b = sqp.tile([P, N], F32R, tag="sq", bufs=2, name="sq4b")
            u_all4 = uwp.tile([P, NT, 256], BF16, tag="uall256", bufs=2, name="ua4")
            w_sb4 = uwp.tile([P, NT, 256], BF16, tag="wall256", bufs=2,
                             name="wsb4")
            lhs4 = [lambda c: base3[0][:, cs(c)], lambda c: base3[1][:, cs(c)],
                    lambda c: auxA4[0:2, cs(c)]]
            rhs4 = [lambda hs: base3[0][:, hs], lambda hs: base3[1][:, hs],
                    lambda hs: auxB4[0:2, hs]]

            def prep4(c):
                psu = pp.tile([P, 256], F32, tag="ps", name=f"psu4_{c}")
                nc.tensor.matmul(psu[:], base3[0][:, cs(c)], uW4s[0][:],
                                 start=True, stop=False)
                nc.tensor.matmul(psu[:], base3[1][:, cs(c)], uW4s[1][:],
                                 start=False, stop=True)
                nc.scalar.copy(out=u_all4[:, c, :], in_=psu[:])
                psw = pp.tile([P, 256], F32, tag="ps", name=f"psw4_{c}")
                nc.tensor.matmul(psw[:], base3[0][:, cs(c)], wW4s[0][:],
                                 start=True, stop=False)
                nc.tensor.matmul(psw[:], base3[1][:, cs(c)], wW4s[1][:],
                                 start=False, stop=False)
                nc.tensor.matmul(psw[:], ones1[:1, cs(c)], wb4s[:],
                                 start=False, stop=True)
                nc.scalar.copy(out=w_sb4[:, c, :], in_=psw[:])
                if c == 3:
                    nc.scalar.square(out=sq4a[:, 0:512], in_=base3[0][:, 0:512])
                    nc.scalar.square(out=sq4b[:, 0:512], in_=base3[1][:, 0:512])
                    build_aux_h([sq4a[:, :], sq4b[:, :]], auxA4, auxB4, 0)
                    store_u_h(u_all4, u4d, 0)
                elif c == NT - 1:
                    store_u_h(u_all4, u4d, 1)
                elif c == 4:
                    early_t0("ec4", lhs4, rhs4)
                elif c == 6:
                    early_t0_mid("ec4", lhs4, rhs4,
                                 [(sq4a[:, :], base3[0][:, :]),
                                  (sq4b[:, :], base3[1][:, :])], auxA4, auxB4)

            idxs3 = [idx3_t0]
            for c in range(NT):
                if c + 1 < NT:
                    idxs3.append(topk20(d3[c + 1]))
                gather_combine(idxs3[c], u3d, w_sb3[:, c, :], 256, c, x3[c], u_all3)
                transpose_into(x3[c][:, 0:P], base3[0], 0, c)
                transpose_into(x3[c][:, P:256], base3[1], 0, c)
                p1_unit(c, 0)
                prep4(c)
            # =====================================================
            # ec4
            # =====================================================
            x4 = [xp.tile([P, 256], BF16, tag=f"x3_{c}", bufs=2, name=f"x4_{c}")
                  for c in range(NT)]
            d4_t0, idx4_t0 = finish_t0("ec4", lhs4, rhs4,
                [(sq4a[:, :], base3[0][:, :]),
                 (sq4b[:, :], base3[1][:, :])], auxA4, auxB4)
            d4 = [d4_t0]
            d4 += dist_tiles(lhs4, rhs4, tiles=range(1, NT))
            base4 = [bp.tile([P, N], F32R, tag=f"base4_{i}", name=f"base4_{i}")
                     for i in range(2)]
            cat = [base3[0], base3[1], base4[0], base4[1]]
            p2t4 = mc.tile([P, 32], F32, tag="ptmp4", name="p2t4")

            def p2q_unit(j, q):
                # quarter-width x_t2 unit (256-wide keeps f32r matmuls at
                # 1 cycle/row; narrower runs at 1/4 rate): quarter q needs
                # only ec4 point tiles 2q, 2q+1, so all but the last
                # quarter overlap the gather phase
                qs = slice(q * 256, (q + 1) * 256)
                pst = pp.tile([P, 256], F32, tag="ps", name=f"pt2_{j}_{q}")
                for ki in range(4):
                    nc.tensor.matmul(pst[:], w5s[ki][:, cs(j)],
                                     cat[ki][:, qs],
                                     start=(ki == 0), stop=(ki == 3))
                nc.vector.tensor_reduce(
                    out=p2t4[:, 4 * j + q:4 * j + q + 1], in_=pst[:],
                    axis=AX.X, op=OP.max)

            fcolb = const.tile([P, 16], BF16)
            ps1 = pp.tile([1, 512], F32, tag="ps1", bufs=1, name="ps1")
            idxs4 = [topk20(d4[0], split_encode=True)]
            for c in range(NT):
                if c + 1 < NT:
                    idxs4.append(topk20(d4[c + 1]))
                gather_combine(idxs4[c], u4d, w_sb4[:, c, :], 256, c, x4[c], u_all4)
                transpose_into(x4[c][:, 0:P], base4[0], 0, c)
                transpose_into(x4[c][:, P:256], base4[1], 0, c)
                if c < 4:
                    # remaining x_t1 units (base1 stays live through ec4)
                    p1_unit(2 * c, 1)
                    p1_unit(2 * c + 1, 1)
                if c in (1, 2):
                    for j in range(4 * (c - 1), 4 * (c - 1) + 4):
                        p2q_unit(j, 0)
                elif c in (3, 4):
                    for j in range(4 * (c - 3), 4 * (c - 3) + 4):
                        p2q_unit(j, 1)
                elif c == 5:
                    for j in range(NT):
                        p2q_unit(j, 2)
                elif c == 6:
                    # PE p-state warm-up: the cost model halves matmul rate
                    # after any PE idle and needs 3us of continuous work to
                    # re-ramp. Keep PE busy through tile 7's gather window
                    # (where it would idle) so the tail's 32 x_t2 matmuls +
                    # classifier matmuls run at full rate, not half.
                    warm = pp.tile([P, 512], F32, tag="ps", name="warm")
                    for wi in range(20):
                        nc.tensor.matmul(warm[:], w5s[0][:, 0:P],
                                         base3[0][:, 0:512],
                                         start=True, stop=True)
                if c == 3:
                    # p1 is complete: fold its half of the classifier's
                    # first layer into the gather phase
                    p1pre = mc.tile([P, 8], F32, tag="ppre", name="p1pre")
                    nc.vector.tensor_reduce(
                        out=p1pre[:],
                        in_=p1tmp[:].rearrange("p (j h) -> p j h", h=2),
                        axis=AX.X, op=OP.max)
                    nc.vector.tensor_add(out=p1pre[:], in0=p1pre[:],
                                         in1=p1bias[:])
                    leaky(fcol[:, 0:8], p1pre[:])
                    nc.scalar.copy(out=fcolb[:, 0:8], in_=fcol[:, 0:8])
                    for j in range(8):
                        nc.tensor.matmul(ps1[:], fcolb[:, j:j + 1], l1ts[j][:],
                                         start=(j == 0), stop=False)
                    for j in range(12, 16):
                        nc.sync.dma_start(out=l1ts[j][:],
                                          in_=l1[j * P:(j + 1) * P, :])

            # =====================================================
            # x_t2 -> p2 tail (last quarter needs point tiles 6-7)
            # =====================================================
            for j in range(NT):
                p2q_unit(j, 3)
            p2pre = mc.tile([P, 8], F32, tag="ppre", name="p2pre")
            nc.vector.tensor_reduce(
                out=p2pre[:], in_=p2t4[:].rearrange("p (j q) -> p j q", q=4),
                axis=AX.X, op=OP.max)
            nc.vector.tensor_add(out=p2pre[:], in0=p2pre[:], in1=p2bias[:])
            leaky(fcol[:, 8:16], p2pre[:])

            # =====================================================
            # final MLP
            # =====================================================
            nc.scalar.copy(out=fcolb[:, 8:16], in_=fcol[:, 8:16])
            for j in range(8, 16):
                nc.tensor.matmul(ps1[:], fcolb[:, j:j + 1], l1ts[j][:],
                                 start=False, stop=False)
            nc.tensor.matmul(ps1[:], ones1[:1, :1], l1bs[:],
                             start=False, stop=True)
            f1sb = mc.tile([1, 512], F32, tag="f1pre", name="f1sb")
            leaky(f1sb[:], ps1[:])
            f2col = mc.tile([P, 4], BF16, tag="f2col", name="f2col")
            for j in range(4):
                pst = pp.tile([P, 1], F32, tag="ps", name=f"ptc1_{j}")
                nc.tensor.transpose(out=pst[:], in_=f1sb[:1, j * P:(j + 1) * P],
                                    identity=ident[:1, :1])
                nc.scalar.copy(out=f2col[:, j:j + 1], in_=pst[:])

            ps2 = pp.tile([1, 256], F32, tag="ps", name="ps2")
            for j in range(4):
                nc.tensor.matmul(ps2[:], f2col[:, j:j + 1], l2ts[j][:],
                                 start=(j == 0), stop=False)
            nc.tensor.matmul(ps2[:], ones1[:1, :1], l2bs[:],
                             start=False, stop=True)
            f2sb = mc.tile([1, 256], F32, tag="f2pre", name="f2sb")
            leaky(f2sb[:], ps2[:])
            f3col = mc.tile([P, 2], BF16, tag="f3col", name="f3col")
            for j in range(2):
                pst = pp.tile([P, 1], F32, tag="ps", name=f"ptc2_{j}")
                nc.tensor.transpose(out=pst[:], in_=f2sb[:1, j * P:(j + 1) * P],
                                    identity=ident[:1, :1])
                nc.scalar.copy(out=f3col[:, j:j + 1], in_=pst[:])

            ps3 = pp.tile([1, 40], F32, tag="ps", name="ps3")
            for j in range(2):
                nc.tensor.matmul(ps3[:], f3col[:, j:j + 1], l3ts[j][:],
                                 start=(j == 0), stop=False)
            nc.tensor.matmul(ps3[:], ones1[:1, :1], l3bs[:],
                             start=False, stop=True)
            osb = mc.tile([1, 40], F32, tag="osb", name="osb")
            nc.scalar.copy(out=osb[:], in_=ps3[:])
            nc.sync.dma_start(out=out_d[:], in_=osb[:])

    _split_excess_waits(nc)
    nc.finalize()
    return nc


def _split_excess_waits(nc, cap=1):
    """Walrus codegen rejects instructions with more than `cap` sem waits
    (matmul LDWEIGHTS allows only 1; most others take 2).
    Hoist the excess onto same-engine NOPs inserted just before."""
    for b in nc.m.functions[0].blocks:
        new = []
        changed = False
        for inst in b.instructions:
            cap = 1
            si = getattr(inst, "sync_info", None)
            if si is not None and si.on_wait is not None and len(si.on_wait) > cap:
                waits = list(si.on_wait)
                rest = waits[cap:]
                k = 0
                while rest:
                    chunk, rest = rest[:cap], rest[cap:]
                    nop = mybir.InstNoOp(name=f"{inst.name}-ws{k}", ins=[],
                                         outs=[])
                    nop.engine = inst.engine
                    nop.sync_info = mybir.SyncInfo(on_wait=chunk, on_update=[])
                    new.append(nop)
                    k += 1
                inst.sync_info = mybir.SyncInfo(on_wait=waits[:cap],
                                                on_update=list(si.on_update))
                changed = True
            new.append(inst)
        if changed:
            b.instructions = new


def _round_f32r(x):
    """Round f32 -> f32r bit pattern (13 explicit mantissa bits, RNE-ish)."""
    x = np.ascontiguousarray(x, np.float32)
    u = x.view(np.uint32).astype(np.uint64)
    r = ((u + (1 << 9)) & np.uint64(0xFFFFFC00)).astype(np.uint32)
    return r.view(np.float32)


def prep_weights(inp):
    """Host-side constant folding: BN scales/biases into weights, EdgeConv
    linear decomposition, transposes into lhsT/rhs layouts."""
    S = 1.0 / math.sqrt(1.0 + 1e-5)
    f = np.float32
    w = {}
    s1 = (inp["g1"] * S).astype(f)
    w["uW1"] = np.ascontiguousarray((s1[:, None] * inp["W1"][:, :3]).T, f)
    w["wW1"] = np.ascontiguousarray(
        (s1[:, None] * (inp["W1"][:, 3:] - inp["W1"][:, :3])).T, f)
    w["wb1"] = inp["b1"][None].astype(f)
    s2 = (inp["g2"] * S).astype(f)
    w["uW2"] = np.ascontiguousarray((s2[:, None] * inp["W2"][:, :64]).T, f)
    w["wW2"] = np.ascontiguousarray(
        (s2[:, None] * (inp["W2"][:, 64:] - inp["W2"][:, :64])).T, f)
    w["wb2"] = inp["b2"][None].astype(f)
    s3 = (inp["g3"] * S).astype(f)
    W3 = inp["W3"]
    Wa3 = W3[:, :256]; Wb3 = W3[:, 256:]
    Wa3e = Wa3[:, :128] + Wa3[:, 128:]
    Wb3e = Wb3[:, :128] + Wb3[:, 128:]
    w["uW3"] = np.ascontiguousarray((s3[:, None] * Wa3e).T, f)
    w["wW3"] = np.ascontiguousarray((s3[:, None] * (Wb3e - Wa3e)).T, f)
    w["wb3"] = inp["b3"][None].astype(f)
    s4 = (inp["g4"] * S).astype(f)
    w["uW4"] = np.ascontiguousarray((s4[:, None] * inp["W4"][:, :256]).T, f)
    w["wW4"] = np.ascontiguousarray(
        (s4[:, None] * (inp["W4"][:, 256:] - inp["W4"][:, :256])).T, f)
    w["wb4"] = inp["b4"][None].astype(f)
    s2m = (inp["g2m"] * S).astype(f)
    w["w2m"] = np.ascontiguousarray((s2m[:, None] * inp["W2m"]).T, f)
    w["w2mb"] = inp["b2m"][None].astype(f)
    s5 = (inp["g5"] * S).astype(f)
    w["w5"] = np.ascontiguousarray((s5[:, None] * inp["W5"]).T, f)
    w["w5b"] = inp["b5"][None].astype(f)
    s6 = (inp["g6"] * S).astype(f)
    w["l1"] = np.ascontiguousarray((s6[:, None] * inp["L1"]).T, f)
    w["l1b"] = inp["b6"][None].astype(f)
    s7 = (inp["g7"] * S).astype(f)
    w["l2"] = np.ascontiguousarray((s7[:, None] * inp["L2"]).T, f)
    w["l2b"] = (s7 * inp["bL2"] + inp["b7"])[None].astype(f)
    w["l3"] = np.ascontiguousarray(inp["L3"].T, f)
    w["l3b"] = inp["bL3"][None].astype(f)
    import ml_dtypes
    out = {}
    for k, v in w.items():
        if k in ("l1", "l2", "l3"):
            out[k] = np.ascontiguousarray(v.astype(ml_dtypes.bfloat16))
        else:
            out[k] = _round_f32r(v)
    return out


_NC_CACHE = None


def get_nc():
    global _NC_CACHE
    if _NC_CACHE is None:
        _NC_CACHE = build_nc()
    return _NC_CACHE


def run(inputs, trace=False):
    nc = get_nc()
    w = prep_weights(inputs)
    x = np.asarray(inputs["x"], np.float32)
    in_maps = [{"xs": _round_f32r(x[i]), **w} for i in range(8)]
    res = run_bass_kernel_spmd(nc, in_maps, core_ids=list(range(8)), trace=trace)
    out = np.concatenate([res.results[i]["out"] for i in range(8)], axis=0)
    return out, res


def kernel(**inputs) -> np.ndarray:
    out, _ = run(inputs)
    return out.astype(np.float32)

